# revision 21
# baseline (speedup 1.0000x reference)
"""Trainium2 Bass kernel for CausalSelfAttention (GQA + QK-RMSNorm + RoPE +
sliding-window causal attention + out-proj), tensor-parallel over 8 NeuronCores.

Sharding: core i owns q heads 4i..4i+3 and kv group i (split of the qkv output
dim and the proj input dim). The QK RMSNorm spans ALL heads (norm over the full
flattened q/k vectors), so per-core partial sums of squares are combined with a
tiny AllReduce. The proj contribution of each core is a partial sum over its
heads; partials are summed on the host.

v3: deferred epilogues (all 4 qkv tiles first, 2 batched AllReduces posted
with ~80us of program-order slack so no engine stream ever wedges on the
collective), 512-col qkv matmuls with c-outer piece-streamed x, den via DVE
pair-folds + one matmul, single-Rsqrt rstd path, vector-engine DMA queue
dedicated to latency-critical small transfers, batched output stores.

Self-contained: hardcodes B=1, T=2048, C=4096, H=32, G=8, D=128, W=1024.
"""

import sys
import types
import numpy as np
import ml_dtypes

import concourse.bass as bass
import concourse.tile as tile
from concourse import bacc, mybir
from concourse import bass_utils
from concourse.bass import ts
from concourse.masks import make_identity

BF16 = ml_dtypes.bfloat16
FP32 = mybir.dt.float32
BF = mybir.dt.bfloat16

T = 2048          # tokens
C = 4096          # n_embd
D = 128           # head dim
HL = 4            # local q heads per core
NCORES = 8
EPS = 1e-5
NT = 4            # t-tiles of 512
NG = 8            # q groups of 256
NKV = 16          # kv blocks of 128
NPIECE = 8        # x pieces per t-tile (4 c-chunks each)


def _install_ntff_hook():
    """Re-register the axon NTFF profiling hook (the image lacks
    antenv.axon_hooks, so boot() degraded silently)."""
    if "antenv.axon_hooks" in sys.modules:
        return
    mod = types.ModuleType("antenv.axon_hooks")
    holder = [None]
    mod.set_axon_ntff_profile_hook = lambda h: holder.__setitem__(0, h)
    mod.get_axon_ntff_profile_hook = lambda: holder[0]
    sys.modules["antenv.axon_hooks"] = mod
    try:
        import antenv
        antenv.axon_hooks = mod
        from trn_agent_boot.trn_boot import _ntff_profile_via_ctypes
        mod.set_axon_ntff_profile_hook(
            _ntff_profile_via_ctypes("/opt/axon/libaxon_pjrt.so"))
    except Exception:
        pass


_install_ntff_hook()


def build_program():
    nc = bacc.Bacc("TRN2", target_bir_lowering=False, debug=False,
                   num_devices=NCORES)

    xt_d = nc.dram_tensor("xt", [4, 128, 32, 512], BF, kind="ExternalInput").ap()
    wq_d = nc.dram_tensor("wq", [128, 32, 768], BF, kind="ExternalInput").ap()
    wp_d = nc.dram_tensor("wp", [128, 4, C], BF, kind="ExternalInput").ap()
    cs_d = nc.dram_tensor("cs", [128, T], BF, kind="ExternalInput").ap()
    sn_d = nc.dram_tensor("sn", [128, T], BF, kind="ExternalInput").ap()
    iw2_d = nc.dram_tensor("iw2", [128, 8], BF, kind="ExternalInput").ap()
    pm_d = nc.dram_tensor("pm", [128, 128], BF, kind="ExternalInput").ap()
    yo_d = nc.dram_tensor("yo", [16, 128, 2, T], BF, kind="ExternalOutput").ap()

    with tile.TileContext(nc) as tc:
        _emit(nc, tc, xt_d, wq_d, wp_d, cs_d, sn_d, iw2_d, pm_d, yo_d)
    nc.compile()
    return nc


def _emit(nc, tc, xt_d, wq_d, wp_d, cs_d, sn_d, iw2_d, pm_d, yo_d):
    import contextlib
    ctx = contextlib.ExitStack()
    with ctx:
        # ---------------- pools ----------------
        const = ctx.enter_context(tc.tile_pool(name="const", bufs=1))
        persist = ctx.enter_context(tc.tile_pool(name="persist", bufs=1))
        xpool = ctx.enter_context(tc.tile_pool(name="xpool", bufs=4))
        qkvpool = ctx.enter_context(tc.tile_pool(name="qkvpool", bufs=4))
        qkvcopy = ctx.enter_context(tc.tile_pool(name="qkvcopy", bufs=1))
        sqpool = ctx.enter_context(tc.tile_pool(name="sqpool", bufs=2))
        stgpool = ctx.enter_context(tc.tile_pool(name="stgpool", bufs=2))
        rcv = ctx.enter_context(tc.tile_pool(name="rcv", bufs=1))
        rqbp = ctx.enter_context(tc.tile_pool(name="rqbp", bufs=1))
        ropep = ctx.enter_context(tc.tile_pool(name="ropep", bufs=1))
        ppool = ctx.enter_context(tc.tile_pool(name="ppool", bufs=13))
        daccp = ctx.enter_context(tc.tile_pool(name="daccp", bufs=3))
        dsbp = ctx.enter_context(tc.tile_pool(name="dsbp", bufs=2))
        drbp = ctx.enter_context(tc.tile_pool(name="drbp", bufs=2))
        yhatp = ctx.enter_context(tc.tile_pool(name="yhatp", bufs=2))
        ostgp = ctx.enter_context(tc.tile_pool(name="ostgp", bufs=2))
        dram = ctx.enter_context(tc.tile_pool(name="dram", bufs=2, space="DRAM"))

        # PSUM budget (8 banks, slots are bank-granular):
        #   mm    6 banks (qkv accumulators, rope-swap, scores, proj)
        #   aux   1 bank  (sumsq rows in qkv phase; den rows in attn phase)
        #   yps   1 bank  (AV accumulator)
        mm = ctx.enter_context(tc.tile_pool(name="mm", bufs=6, space="PSUM"))
        aux = ctx.enter_context(tc.tile_pool(name="aux", bufs=1, space="PSUM"))
        yps = ctx.enter_context(tc.tile_pool(name="yps", bufs=1, space="PSUM"))

        # ---------------- warmup collective ----------------
        # The first collective of the program absorbs ~100-140us of CC-stream
        # init / cross-core launch skew. Pay that cost on a 32-byte dummy
        # issued at t~0 (hidden under the qkv phase) so the real AllReduces
        # run at their ~18us steady-state latency.
        zrow = const.tile([1, 8], FP32)
        nc.vector.memset(zrow, 0.0)
        dmy_in = dram.tile([1, 8], FP32, tag="dmyi")
        dmy_out = dram.tile([1, 8], FP32, tag="dmyo")
        nc.gpsimd.dma_start(dmy_in, zrow)
        nc.gpsimd.collective_compute(
            "AllReduce", mybir.AluOpType.add,
            replica_groups=[list(range(NCORES))],
            ins=[dmy_in.opt()], outs=[dmy_out.opt()])

        # ---------------- constants (no DMA deps first) ----------------
        ident = const.tile([128, 128], BF)
        make_identity(nc, ident)
        ones = const.tile([128, 1], BF)
        nc.vector.memset(ones, 1.0)
        zerob = const.tile([128, 1], FP32)
        nc.vector.memset(zerob, 0.0)
        # sqrt bias: k rows get 128*eps (1/sqrt(d) folded into the k sumsq
        # via the host iw2 weighting)
        epsq = const.tile([1, 1], FP32)
        nc.vector.memset(epsq, EPS)
        epsk = const.tile([1, 1], FP32)
        nc.vector.memset(epsk, 128.0 * EPS)
        # window/causal edge masks, [kv=128, 2*256] (a pair of kv blocks for
        # the same 256 queries). diag pair: m in {0,1}; tail pair: m in {-8,-7}
        # where m = j - 2g. allowed iff 0 <= d <= 1023 with d = q - kv - 128*m.
        mask_diag = const.tile([128, 512], BF)
        nc.vector.memset(mask_diag, 1.0)
        mask_tail = const.tile([128, 512], BF)
        nc.vector.memset(mask_tail, 1.0)
        for half, m in ((0, 0), (1, 1)):
            nc.gpsimd.affine_select(
                out=mask_diag[:, 256 * half:256 * half + 256],
                in_=mask_diag[:, 256 * half:256 * half + 256],
                compare_op=mybir.AluOpType.is_ge,
                fill=0.0, base=-128 * m, pattern=[[1, 256]],
                channel_multiplier=-1)
        for half, m in ((0, -8), (1, -7)):
            nc.gpsimd.affine_select(
                out=mask_tail[:, 256 * half:256 * half + 256],
                in_=mask_tail[:, 256 * half:256 * half + 256],
                compare_op=mybir.AluOpType.is_ge,
                fill=0.0, base=128 * m + 1023, pattern=[[-1, 256]],
                channel_multiplier=1)

        # small input consts on sync queue (before wq so they land early)
        iw2 = const.tile([128, 8], BF)
        nc.sync.dma_start(iw2, iw2_d)
        pm = const.tile([128, 128], BF)
        nc.sync.dma_start(pm, pm_d)

        # ---------------- persistent buffers ----------------
        # wq on the sync DMA queue; x pieces go on the scalar queue so the
        # gpsimd sequencer (which hosts the collective triggers) stays empty.
        wq_sb = persist.tile([128, 32, 768], BF)
        for h in range(8):
            nc.sync.dma_start(wq_sb[:, ts(h, 4), :], wq_d[:, ts(h, 4), :])
        wp_sb = persist.tile([128, 4, C], BF)   # loaded later (deferred)

        # cos/sin full tables early on the gpsimd queue (it is otherwise idle
        # until the first collective trigger)
        cs_sb = persist.tile([128, T], BF)
        nc.gpsimd.dma_start(cs_sb, cs_d)
        sn_sb = persist.tile([128, T], BF)
        nc.gpsimd.dma_start(sn_sb, sn_d)

        qhat = persist.tile([128, HL, T], BF)     # roped+normed q
        khat = persist.tile([128, T], BF)         # roped+normed k (1/sqrt(d) in)
        v_sb = persist.tile([128, NKV, 128], BF)  # v transposed [kv, d]

        qkv_tiles = {}
        arins = {}
        arouts = {}
        # AR batches: tiles (0,1) together, then 2 and 3 alone so their rstd
        # unblocks as soon as that tile's sumsq lands
        BATCHES = ((0, 1), (2,), (3,))
        TILE_BATCH = {0: (0, 0), 1: (0, 1), 2: (1, 0), 3: (2, 0)}

        # ========= qkv tile: c-outer, 6 open PSUM accumulators =========
        def emit_qkv(t):
            qkvt = qkvpool.tile([128, 5, 512], BF, tag="qkvt")
            vstage = qkvcopy.tile([128, 512], BF, tag="vstage")
            qkv_tiles[t] = (qkvt, vstage)
            ps = []
            for _o in range(6):
                pso = mm.tile([128, 512], FP32, tag="mm")
                ps.append(pso)
            for p in range(NPIECE):
                xp = xpool.tile([128, 4, 512], BF, tag="x")
                nc.scalar.dma_start(xp, xt_d[t, :, ts(p, 4), :])
                for o in range(6):
                    for ci in range(4):
                        c = 4 * p + ci
                        nc.tensor.matmul(ps[o],
                                         wq_sb[:, c, ts(o, 128)],
                                         xp[:, ci, :],
                                         start=(c == 0), stop=(c == 31))
            for o in range(6):
                if o < 5:
                    nc.scalar.copy(qkvt[:, o, :], ps[o])
                else:
                    nc.scalar.copy(vstage, ps[o])

        # ========= epilogue: v transpose + weighted sumsq + AR send =========
        def emit_epilogue(t):
            qkvt, vstage = qkv_tiles[t]
            # v transpose: [d, kv] -> [kv, d]
            for bb in range(4):
                tps = mm.tile([128, 128], BF, tag="mm")
                nc.tensor.transpose(tps, vstage[:, ts(bb, 128)], ident)
                nc.vector.tensor_copy(v_sb[:, 4 * t + bb, :], tps)
            # sum of squares (weighted by 1/w^2; k row weighted 4*128x on host:
            # 4x so q/k share the 1/4096 mean divisor, 128x folds 1/sqrt(d)
            # into rstd_k). row 0 = q, row 32 = k.
            sums = aux.tile([33, 512], FP32, tag="aux")
            for cch in range(4):
                sq = sqpool.tile([128, 512], BF, tag="sq")
                nc.vector.tensor_mul(sq, qkvt[:, cch, :], qkvt[:, cch, :])
                nc.tensor.matmul(sums[0:1, :], iw2[:, cch:cch + 1], sq,
                                 start=(cch == 0), stop=(cch == 3))
            sqk = sqpool.tile([128, 512], BF, tag="sq")
            nc.vector.tensor_mul(sqk, qkvt[:, 4, :], qkvt[:, 4, :])
            nc.tensor.matmul(sums[32:33, :], iw2[:, 4:5], sqk,
                             start=True, stop=True)
            stg = stgpool.tile([33, 512], FP32, tag="stg")
            nc.vector.tensor_copy(stg[0:1, :], sums[0:1, :])
            nc.vector.tensor_copy(stg[32:33, :], sums[32:33, :])
            # stage rows [q(t), k(t)] of this AR batch's input
            b, slot = TILE_BATCH[t]
            nrows = 2 * len(BATCHES[b])
            if slot == 0:
                arin = dram.tile([nrows, 512], FP32, tag=f"arin{b}")
                arout = dram.tile([nrows, 512], FP32, tag=f"arout{b}")
                arins[b] = arin
                arouts[b] = arout
            # AR sends ride the sync queue: its sequencer is idle mid-kernel,
            # and the wp loads emitted between sends are not latency-critical
            nc.sync.dma_start(arins[b][2 * slot:2 * slot + 1, :], stg[0:1, :])
            nc.sync.dma_start(arins[b][2 * slot + 1:2 * slot + 2, :],
                              stg[32:33, :])

        def emit_allreduce(b):
            nc.gpsimd.collective_compute(
                "AllReduce", mybir.AluOpType.add,
                replica_groups=[list(range(NCORES))],
                ins=[arins[b].opt()], outs=[arouts[b].opt()])

        # ========= ep2: rstd + rope for one AR batch =========
        rqbs = {}
        rkbs = {}

        def emit_rstd(b):
            # rstd = 1/sqrt(ss/4096 + eps); k rows come host-scaled by 128 so
            # this is rstd_k/sqrt(d) with bias 128*eps. (Rsqrt ACT is
            # blocklisted, so Sqrt + the DVE fast-NR reciprocal. Per-row
            # [1,512] tiles: SBUF APs must start at partition 0/32/64/96.)
            for slot, t in enumerate(BATCHES[b]):
                rsq = rcv.tile([1, 512], FP32, tag="rsq")
                nc.gpsimd.dma_start(rsq, arouts[b][2 * slot:2 * slot + 1, :])
                rsk = rcv.tile([1, 512], FP32, tag="rsk")
                nc.gpsimd.dma_start(rsk,
                                    arouts[b][2 * slot + 1:2 * slot + 2, :])
                nc.scalar.activation(rsq, rsq,
                                     mybir.ActivationFunctionType.Sqrt,
                                     bias=epsq, scale=1.0 / 4096.0)
                nc.scalar.activation(rsk, rsk,
                                     mybir.ActivationFunctionType.Sqrt,
                                     bias=epsk, scale=1.0 / 4096.0)
                nc.vector.reciprocal_approx_fast(rsq, rsq)
                nc.vector.reciprocal_approx_fast(rsk, rsk)
                rqb = rqbp.tile([128, 512], FP32, tag=f"rqb{t % 2}")
                nc.gpsimd.partition_broadcast(rqb, rsq)
                rkb = rqbp.tile([128, 512], FP32, tag=f"rkb{t % 2}")
                nc.gpsimd.partition_broadcast(rkb, rsk)
                rqbs[t] = rqb
                rkbs[t] = rkb

        def emit_rope(t):
            qkvt, _ = qkv_tiles.pop(t)
            rqb = rqbs.pop(t)
            rkb = rkbs.pop(t)
            tc_sl = ts(t, 512)
            for h in range(5):  # 4 q heads + k
                swps = mm.tile([128, 512], FP32, tag="mm")
                nc.tensor.matmul(swps, pm, qkvt[:, h, :],
                                 start=True, stop=True)
                t1 = ropep.tile([128, 512], BF, tag="t1")
                t2 = ropep.tile([128, 512], BF, tag="t2")
                nc.vector.tensor_mul(t1, qkvt[:, h, :], cs_sb[:, tc_sl])
                nc.vector.tensor_mul(t2, swps, sn_sb[:, tc_sl])
                nc.vector.tensor_add(t1, t1, t2)
                if h < 4:
                    nc.vector.tensor_mul(qhat[:, h, tc_sl], t1, rqb)
                else:
                    nc.vector.tensor_mul(khat[:, tc_sl], t1, rkb)

        # ========= attention =========
        # Per step: scores+exp+mask+den-fold for (g,h) interleaved with AV
        # matmuls of the previous head at pair granularity so the PE always
        # has ready work adjacent to each exp-gated score pair.
        yhats = {}
        pending = []   # one staged (g, h, pts, dacc) awaiting AV emission

        def emit_da_tail(g, h, yp, dacc):
            # den matmul (single, over the pair-folded sums) then 1/den on
            # DVE (custom NR op; SBUF operands) and the yhat scale.
            dp = aux.tile([1, 256], FP32, tag="aux")
            nc.tensor.matmul(dp, ones[:, 0:1], dacc, start=True, stop=True)
            yhat = yhats[g // 2]
            gc = slice(256 * (g % 2), 256 * (g % 2) + 256)
            ld = dsbp.tile([1, 256], FP32, tag="ld")
            nc.scalar.copy(ld, dp)
            rc = dsbp.tile([1, 256], FP32, tag="rc")
            nc.vector.reciprocal_approx_fast(rc, ld)
            drb = drbp.tile([128, 256], FP32, tag="drb")
            nc.gpsimd.partition_broadcast(drb, rc)
            nc.vector.tensor_mul(yhat[:, h, gc], yp, drb)

        def attn_step(g, h):
            """Emit scores for (g,h) [if not None] + AV for pending."""
            cur = []
            dacc = None
            npair_cur = 0
            if g is not None:
                jlo = max(0, 2 * g - 8)
                jhi = 2 * g + 1
                cur_js = list(range(jlo, jhi, 2))
                npair_cur = len(cur_js)
                dacc = daccp.tile([128, 256], BF, tag="dacc")
            prev = pending.pop(0) if (len(pending) >= 2 or
                                      (g is None and pending)) else None
            if prev is not None:
                pg, ph, ppts, pdacc = prev
                pjlo = ppts[0][0]
                pjhi = ppts[-1][0] + 1
                yp = yps.tile([128, 256], FP32, tag="y")
            nsteps = max(npair_cur, len(ppts) if prev else 0)
            for idx in range(nsteps):
                if idx < npair_cur:
                    j = cur_js[idx]
                    sc = mm.tile([128, 512], FP32, tag="mm")
                    nc.tensor.matmul(sc[:, 0:256], khat[:, ts(j, 128)],
                                     qhat[:, h, ts(g, 256)],
                                     start=True, stop=True)
                    nc.tensor.matmul(sc[:, 256:512], khat[:, ts(j + 1, 128)],
                                     qhat[:, h, ts(g, 256)],
                                     start=True, stop=True)
                    pt = ppool.tile([128, 512], BF, tag="p")
                    nc.scalar.activation(pt, sc,
                                         mybir.ActivationFunctionType.Exp,
                                         bias=zerob)
                    if j == 2 * g:
                        nc.vector.tensor_mul(pt, pt, mask_diag)
                    elif j == jlo and g >= 4:
                        nc.vector.tensor_mul(pt, pt, mask_tail)
                    # den pair-fold on DVE
                    if idx == 0:
                        nc.vector.tensor_add(dacc, pt[:, 0:256], pt[:, 256:512])
                    else:
                        nc.vector.tensor_add(dacc, dacc, pt[:, 0:256])
                        nc.vector.tensor_add(dacc, dacc, pt[:, 256:512])
                    cur.append((j, pt))
                if prev is not None and idx < len(ppts):
                    j, pt = ppts[idx]
                    for k in range(2):
                        jj = j + k
                        sl = slice(256 * k, 256 * k + 256)
                        nc.tensor.matmul(yp, v_sb[:, jj, :], pt[:, sl],
                                         start=(jj == pjlo), stop=(jj == pjhi))
            if prev is not None:
                emit_da_tail(pg, ph, yp, pdacc)
            if g is not None:
                pending.append((g, h, cur, dacc))

        def attn_group(g):
            if g % 2 == 0:
                yhat = yhatp.tile([128, HL, 512], BF, tag="yhat")
                yhats[g // 2] = yhat
            for h in range(HL):
                attn_step(g, h)

        def flush_attn():
            attn_step(None, None)
            attn_step(None, None)

        # ========= out-proj =========
        def emit_proj(p):
            yhat = yhats.pop(p)
            ostg = None
            for o in range(32):
                if o % 2 == 0:
                    ostg = ostgp.tile([128, 2, 512], BF, tag="ostg")
                ps = mm.tile([128, 512], FP32, tag="mm")
                for cch in range(4):
                    nc.tensor.matmul(ps, wp_sb[:, cch, ts(o, 128)],
                                     yhat[:, cch, :],
                                     start=(cch == 0), stop=(cch == 3))
                if o % 2 == 0:
                    nc.scalar.copy(ostg[:, 0, :], ps)
                else:
                    nc.vector.tensor_copy(ostg[:, 1, :], ps)
                    deng = (nc.sync, nc.scalar)[(o // 2) % 2]
                    deng.dma_start(yo_d[o // 2, :, :, ts(p, 512)], ostg)

        # ========= emission =========
        emit_qkv(0)
        emit_epilogue(0)
        emit_qkv(1)
        emit_epilogue(1)
        emit_allreduce(0)
        emit_qkv(2)
        emit_epilogue(2)
        emit_rstd(0)
        emit_rope(0)
        emit_rope(1)
        # wp load here: DMA queues run ahead of the PE, so any earlier and
        # these 4.2MB contend with wq/x during the startup-critical window
        nc.sync.dma_start(wp_sb[:, 0, :], wp_d[:, 0, :])
        nc.sync.dma_start(wp_sb[:, 1, :], wp_d[:, 1, :])
        emit_qkv(3)
        emit_epilogue(3)
        nc.sync.dma_start(wp_sb[:, 2, :], wp_d[:, 2, :])
        nc.sync.dma_start(wp_sb[:, 3, :], wp_d[:, 3, :])
        emit_allreduce(1)
        attn_group(0)
        attn_group(1)
        emit_rstd(1)
        attn_group(2)
        attn_group(3)
        emit_rope(2)
        emit_allreduce(2)
        emit_rstd(2)
        emit_rope(3)
        emit_proj(0)
        attn_group(4)
        emit_proj(1)
        attn_group(5)
        attn_group(6)
        emit_proj(2)
        attn_group(7)
        flush_attn()
        emit_proj(3)


_PROGRAM = None


def _get_program():
    global _PROGRAM
    if _PROGRAM is None:
        _PROGRAM = build_program()
    return _PROGRAM


def make_in_maps(x, cos, sin, W_qkv, norm_q_w, norm_k_w, W_proj):
    x2 = np.asarray(x, np.float32).reshape(T, C)
    xt = np.ascontiguousarray(
        x2.T.reshape(32, 128, 4, 512).transpose(2, 1, 0, 3)).astype(BF16)
    cs = np.ascontiguousarray(np.asarray(cos, np.float32).T).astype(BF16)
    sn_f = np.asarray(sin, np.float32).T.copy()  # [128, T]
    sn_f[0:64, :] *= -1.0  # rotate-half: lower half gets -x2*sin
    sn = np.ascontiguousarray(sn_f).astype(BF16)
    Wq = np.asarray(W_qkv, np.float32)
    Wp = np.asarray(W_proj, np.float32)
    nqw = np.asarray(norm_q_w, np.float32)
    nkw = np.asarray(norm_k_w, np.float32)
    pm = np.zeros((128, 128), np.float32)
    pm[np.arange(128), (np.arange(128) + 64) % 128] = 1.0
    pm = pm.astype(BF16)
    in_maps = []
    for i in range(NCORES):
        qs = slice(512 * i, 512 * i + 512)
        ks = slice(4096 + 128 * i, 4096 + 128 * i + 128)
        vs = slice(5120 + 128 * i, 5120 + 128 * i + 128)
        wq_rows = np.concatenate([
            Wq[qs] * nqw[qs][:, None],
            Wq[ks] * nkw[128 * i:128 * i + 128][:, None],
            Wq[vs],
        ], axis=0)  # [768, 4096]
        wq_t = np.ascontiguousarray(
            wq_rows.T.reshape(32, 128, 768).transpose(1, 0, 2)).astype(BF16)
        wp_t = np.ascontiguousarray(
            Wp[:, 512 * i:512 * i + 512].T.reshape(4, 128, C)
            .transpose(1, 0, 2)).astype(BF16)
        iw2 = np.ones((128, 8), np.float32)
        qw = nqw[qs].reshape(4, 128).T  # [p, chunk]
        kw = nkw[128 * i:128 * i + 128]
        with np.errstate(divide="ignore"):
            iw2[:, 0:4] = np.where(qw != 0.0, qw, 1.0) ** -2.0
            # k row weighted 4x (so q and k share the 1/4096 mean divisor)
            # and a further 128x (folds 1/sqrt(d) into rstd_k)
            iw2[:, 4] = 512.0 * np.where(kw != 0.0, kw, 1.0) ** -2.0
        in_maps.append({
            "xt": xt, "cs": cs, "sn": sn,
            "wq": wq_t, "wp": wp_t, "iw2": iw2.astype(BF16), "pm": pm,
        })
    return in_maps


def combine_outputs(results):
    acc = np.zeros((16, 2, 128, T), np.float32)
    for r in results:
        acc += np.asarray(r["yo"], dtype=np.float32).transpose(0, 2, 1, 3)
    return np.ascontiguousarray(acc.reshape(C, T).T).reshape(1, T, C)


def kernel(x, cos, sin, W_qkv, norm_q_w, norm_k_w, W_proj):
    nc = _get_program()
    in_maps = make_in_maps(x, cos, sin, W_qkv, norm_q_w, norm_k_w, W_proj)
    res = bass_utils.run_bass_kernel_spmd(nc, in_maps,
                                          core_ids=list(range(NCORES)))
    return combine_outputs(res.results)


# revision 22
# speedup vs baseline: 1.0384x; 1.0384x over previous
"""Trainium2 Bass kernel for CausalSelfAttention (GQA + QK-RMSNorm + RoPE +
sliding-window causal attention + out-proj), tensor-parallel over 8 NeuronCores.

Sharding: core i owns q heads 4i..4i+3 and kv group i (split of the qkv output
dim and the proj input dim). The QK RMSNorm spans ALL heads (norm over the full
flattened q/k vectors), so per-core partial sums of squares are combined with a
tiny AllReduce. The proj contribution of each core is a partial sum over its
heads; partials are summed on the host.

v3: deferred epilogues (all 4 qkv tiles first, 2 batched AllReduces posted
with ~80us of program-order slack so no engine stream ever wedges on the
collective), 512-col qkv matmuls with c-outer piece-streamed x, den via DVE
pair-folds + one matmul, single-Rsqrt rstd path, vector-engine DMA queue
dedicated to latency-critical small transfers, batched output stores.

Self-contained: hardcodes B=1, T=2048, C=4096, H=32, G=8, D=128, W=1024.
"""

import sys
import types
import numpy as np
import ml_dtypes

import concourse.bass as bass
import concourse.tile as tile
from concourse import bacc, mybir
from concourse import bass_utils
from concourse.bass import ts
from concourse.masks import make_identity

BF16 = ml_dtypes.bfloat16
FP32 = mybir.dt.float32
BF = mybir.dt.bfloat16

T = 2048          # tokens
C = 4096          # n_embd
D = 128           # head dim
HL = 4            # local q heads per core
NCORES = 8
EPS = 1e-5
NT = 4            # t-tiles of 512
NG = 8            # q groups of 256
NKV = 16          # kv blocks of 128
NPIECE = 8        # x pieces per t-tile (4 c-chunks each)


def _install_ntff_hook():
    """Re-register the axon NTFF profiling hook (the image lacks
    antenv.axon_hooks, so boot() degraded silently)."""
    if "antenv.axon_hooks" in sys.modules:
        return
    mod = types.ModuleType("antenv.axon_hooks")
    holder = [None]
    mod.set_axon_ntff_profile_hook = lambda h: holder.__setitem__(0, h)
    mod.get_axon_ntff_profile_hook = lambda: holder[0]
    sys.modules["antenv.axon_hooks"] = mod
    try:
        import antenv
        antenv.axon_hooks = mod
        from trn_agent_boot.trn_boot import _ntff_profile_via_ctypes
        mod.set_axon_ntff_profile_hook(
            _ntff_profile_via_ctypes("/opt/axon/libaxon_pjrt.so"))
    except Exception:
        pass


_install_ntff_hook()


def build_program():
    nc = bacc.Bacc("TRN2", target_bir_lowering=False, debug=False,
                   num_devices=NCORES)

    xt_d = nc.dram_tensor("xt", [4, 128, 32, 512], BF, kind="ExternalInput").ap()
    wq_d = nc.dram_tensor("wq", [128, 32, 768], BF, kind="ExternalInput").ap()
    wp_d = nc.dram_tensor("wp", [128, 4, C], BF, kind="ExternalInput").ap()
    cs_d = nc.dram_tensor("cs", [128, T], BF, kind="ExternalInput").ap()
    sn_d = nc.dram_tensor("sn", [128, T], BF, kind="ExternalInput").ap()
    iw2_d = nc.dram_tensor("iw2", [128, 8], BF, kind="ExternalInput").ap()
    pm_d = nc.dram_tensor("pm", [128, 128], BF, kind="ExternalInput").ap()
    yo_d = nc.dram_tensor("yo", [16, 128, 2, T], BF, kind="ExternalOutput").ap()

    with tile.TileContext(nc) as tc:
        _emit(nc, tc, xt_d, wq_d, wp_d, cs_d, sn_d, iw2_d, pm_d, yo_d)
    nc.compile()
    return nc


def _emit(nc, tc, xt_d, wq_d, wp_d, cs_d, sn_d, iw2_d, pm_d, yo_d):
    import contextlib
    ctx = contextlib.ExitStack()
    with ctx:
        # ---------------- pools ----------------
        const = ctx.enter_context(tc.tile_pool(name="const", bufs=1))
        persist = ctx.enter_context(tc.tile_pool(name="persist", bufs=1))
        xpool = ctx.enter_context(tc.tile_pool(name="xpool", bufs=5))
        qkvpool = ctx.enter_context(tc.tile_pool(name="qkvpool", bufs=4))
        qkvcopy = ctx.enter_context(tc.tile_pool(name="qkvcopy", bufs=2))
        sqpool = ctx.enter_context(tc.tile_pool(name="sqpool", bufs=2))
        stgpool = ctx.enter_context(tc.tile_pool(name="stgpool", bufs=2))
        rcv = ctx.enter_context(tc.tile_pool(name="rcv", bufs=1))
        rqbp = ctx.enter_context(tc.tile_pool(name="rqbp", bufs=1))
        ropep = ctx.enter_context(tc.tile_pool(name="ropep", bufs=1))
        ppool = ctx.enter_context(tc.tile_pool(name="ppool", bufs=9))
        daccp = ctx.enter_context(tc.tile_pool(name="daccp", bufs=2))
        dsbp = ctx.enter_context(tc.tile_pool(name="dsbp", bufs=2))
        drbp = ctx.enter_context(tc.tile_pool(name="drbp", bufs=2))
        yhatp = ctx.enter_context(tc.tile_pool(name="yhatp", bufs=2))
        ostgp = ctx.enter_context(tc.tile_pool(name="ostgp", bufs=2))
        dram = ctx.enter_context(tc.tile_pool(name="dram", bufs=2, space="DRAM"))

        # PSUM budget (8 banks, slots are bank-granular):
        #   mm    6 banks (qkv accumulators, rope-swap, scores, proj)
        #   aux   1 bank  (sumsq rows in qkv phase; den rows in attn phase)
        #   yps   1 bank  (AV accumulator)
        mm = ctx.enter_context(tc.tile_pool(name="mm", bufs=6, space="PSUM"))
        aux = ctx.enter_context(tc.tile_pool(name="aux", bufs=1, space="PSUM"))
        yps = ctx.enter_context(tc.tile_pool(name="yps", bufs=1, space="PSUM"))

        # ---------------- warmup collective ----------------
        # The first collective of the program absorbs ~100-140us of CC-stream
        # init / cross-core launch skew. Pay that cost on a 32-byte dummy
        # issued at t~0 (hidden under the qkv phase) so the real AllReduces
        # run at their ~18us steady-state latency.
        zrow = const.tile([1, 8], FP32)
        nc.vector.memset(zrow, 0.0)
        dmy_in = dram.tile([1, 8], FP32, tag="dmyi")
        dmy_out = dram.tile([1, 8], FP32, tag="dmyo")
        nc.gpsimd.dma_start(dmy_in, zrow)
        nc.gpsimd.collective_compute(
            "AllReduce", mybir.AluOpType.add,
            replica_groups=[list(range(NCORES))],
            ins=[dmy_in.opt()], outs=[dmy_out.opt()])

        # ---------------- constants (no DMA deps first) ----------------
        ident = const.tile([128, 128], BF)
        make_identity(nc, ident)
        ones = const.tile([128, 1], BF)
        nc.vector.memset(ones, 1.0)
        zerob = const.tile([128, 1], FP32)
        nc.vector.memset(zerob, 0.0)
        # sqrt bias: k rows get 128*eps (1/sqrt(d) folded into the k sumsq
        # via the host iw2 weighting)
        epsq = const.tile([1, 1], FP32)
        nc.vector.memset(epsq, EPS)
        epsk = const.tile([1, 1], FP32)
        nc.vector.memset(epsk, 128.0 * EPS)
        # window/causal edge masks, [kv=128, 2*256] (a pair of kv blocks for
        # the same 256 queries). diag pair: m in {0,1}; tail pair: m in {-8,-7}
        # where m = j - 2g. allowed iff 0 <= d <= 1023 with d = q - kv - 128*m.
        mask_diag = const.tile([128, 512], BF)
        nc.vector.memset(mask_diag, 1.0)
        mask_tail = const.tile([128, 512], BF)
        nc.vector.memset(mask_tail, 1.0)
        for half, m in ((0, 0), (1, 1)):
            nc.gpsimd.affine_select(
                out=mask_diag[:, 256 * half:256 * half + 256],
                in_=mask_diag[:, 256 * half:256 * half + 256],
                compare_op=mybir.AluOpType.is_ge,
                fill=0.0, base=-128 * m, pattern=[[1, 256]],
                channel_multiplier=-1)
        for half, m in ((0, -8), (1, -7)):
            nc.gpsimd.affine_select(
                out=mask_tail[:, 256 * half:256 * half + 256],
                in_=mask_tail[:, 256 * half:256 * half + 256],
                compare_op=mybir.AluOpType.is_ge,
                fill=0.0, base=128 * m + 1023, pattern=[[-1, 256]],
                channel_multiplier=1)

        # small input consts on sync queue (before wq so they land early)
        iw2 = const.tile([128, 8], BF)
        nc.sync.dma_start(iw2, iw2_d)
        pm = const.tile([128, 128], BF)
        nc.sync.dma_start(pm, pm_d)

        # ---------------- persistent buffers ----------------
        # wq on the sync DMA queue; x pieces go on the scalar queue so the
        # gpsimd sequencer (which hosts the collective triggers) stays empty.
        wq_sb = persist.tile([128, 32, 768], BF)
        for h in range(8):
            nc.sync.dma_start(wq_sb[:, ts(h, 4), :], wq_d[:, ts(h, 4), :])
        wp_sb = persist.tile([128, 4, C], BF)   # loaded later (deferred)

        # cos/sin full tables early on the gpsimd queue (it is otherwise idle
        # until the first collective trigger)
        cs_sb = persist.tile([128, T], BF)
        nc.gpsimd.dma_start(cs_sb, cs_d)
        sn_sb = persist.tile([128, T], BF)
        nc.gpsimd.dma_start(sn_sb, sn_d)

        qhat = persist.tile([128, HL, T], BF)     # roped+normed q
        khat = persist.tile([128, T], BF)         # roped+normed k (1/sqrt(d) in)
        v_sb = persist.tile([128, NKV, 128], BF)  # v transposed [kv, d]

        qkv_tiles = {}
        arins = {}
        arouts = {}
        # AR batches: tiles (0,1) together, then 2 and 3 alone so their rstd
        # unblocks as soon as that tile's sumsq lands
        BATCHES = ((0, 1), (2,), (3,))
        TILE_BATCH = {0: (0, 0), 1: (0, 1), 2: (1, 0), 3: (2, 0)}

        # ========= qkv tile: c-outer, 6 open PSUM accumulators =========
        def emit_qkv(t):
            qkvt = qkvpool.tile([128, 5, 512], BF, tag="qkvt")
            vstage = qkvcopy.tile([128, 512], BF, tag="vstage")
            qkv_tiles[t] = (qkvt, vstage)
            ps = []
            for _o in range(6):
                pso = mm.tile([128, 512], FP32, tag="mm")
                ps.append(pso)
            for p in range(NPIECE):
                xp = xpool.tile([128, 4, 512], BF, tag="x")
                nc.scalar.dma_start(xp, xt_d[t, :, ts(p, 4), :])
                for o in range(6):
                    for ci in range(4):
                        c = 4 * p + ci
                        nc.tensor.matmul(ps[o],
                                         wq_sb[:, c, ts(o, 128)],
                                         xp[:, ci, :],
                                         start=(c == 0), stop=(c == 31))
            for o in range(6):
                if o < 5:
                    nc.scalar.copy(qkvt[:, o, :], ps[o])
                else:
                    nc.scalar.copy(vstage, ps[o])

        # ========= epilogue: v transpose + weighted sumsq + AR send =========
        def emit_epilogue(t):
            qkvt, vstage = qkv_tiles[t]
            # v transpose: [d, kv] -> [kv, d]
            for bb in range(4):
                tps = mm.tile([128, 128], BF, tag="mm")
                nc.tensor.transpose(tps, vstage[:, ts(bb, 128)], ident)
                nc.vector.tensor_copy(v_sb[:, 4 * t + bb, :], tps)
            # sum of squares (weighted by 1/w^2; k row weighted 4*128x on host:
            # 4x so q/k share the 1/4096 mean divisor, 128x folds 1/sqrt(d)
            # into rstd_k). row 0 = q, row 32 = k.
            sums = aux.tile([33, 512], FP32, tag="aux")
            for cch in range(4):
                sq = sqpool.tile([128, 512], BF, tag="sq")
                nc.vector.tensor_mul(sq, qkvt[:, cch, :], qkvt[:, cch, :])
                nc.tensor.matmul(sums[0:1, :], iw2[:, cch:cch + 1], sq,
                                 start=(cch == 0), stop=(cch == 3))
            sqk = sqpool.tile([128, 512], BF, tag="sq")
            nc.vector.tensor_mul(sqk, qkvt[:, 4, :], qkvt[:, 4, :])
            nc.tensor.matmul(sums[32:33, :], iw2[:, 4:5], sqk,
                             start=True, stop=True)
            stg = stgpool.tile([33, 512], FP32, tag="stg")
            nc.vector.tensor_copy(stg[0:1, :], sums[0:1, :])
            nc.vector.tensor_copy(stg[32:33, :], sums[32:33, :])
            # stage rows [q(t), k(t)] of this AR batch's input
            b, slot = TILE_BATCH[t]
            nrows = 2 * len(BATCHES[b])
            if slot == 0:
                arin = dram.tile([nrows, 512], FP32, tag=f"arin{b}")
                arout = dram.tile([nrows, 512], FP32, tag=f"arout{b}")
                arins[b] = arin
                arouts[b] = arout
            # AR sends ride the sync queue: its sequencer is idle mid-kernel,
            # and the wp loads emitted between sends are not latency-critical
            nc.sync.dma_start(arins[b][2 * slot:2 * slot + 1, :], stg[0:1, :])
            nc.sync.dma_start(arins[b][2 * slot + 1:2 * slot + 2, :],
                              stg[32:33, :])

        def emit_allreduce(b):
            nc.gpsimd.collective_compute(
                "AllReduce", mybir.AluOpType.add,
                replica_groups=[list(range(NCORES))],
                ins=[arins[b].opt()], outs=[arouts[b].opt()])

        # ========= ep2: rstd + rope for one AR batch =========
        rqbs = {}
        rkbs = {}

        def emit_rstd(b):
            # rstd = 1/sqrt(ss/4096 + eps); k rows come host-scaled by 128 so
            # this is rstd_k/sqrt(d) with bias 128*eps. (Rsqrt ACT is
            # blocklisted, so Sqrt + the DVE fast-NR reciprocal. Per-row
            # [1,512] tiles: SBUF APs must start at partition 0/32/64/96.)
            for slot, t in enumerate(BATCHES[b]):
                rsq = rcv.tile([1, 512], FP32, tag="rsq")
                nc.gpsimd.dma_start(rsq, arouts[b][2 * slot:2 * slot + 1, :])
                rsk = rcv.tile([1, 512], FP32, tag="rsk")
                nc.gpsimd.dma_start(rsk,
                                    arouts[b][2 * slot + 1:2 * slot + 2, :])
                nc.scalar.activation(rsq, rsq,
                                     mybir.ActivationFunctionType.Sqrt,
                                     bias=epsq, scale=1.0 / 4096.0)
                nc.scalar.activation(rsk, rsk,
                                     mybir.ActivationFunctionType.Sqrt,
                                     bias=epsk, scale=1.0 / 4096.0)
                nc.vector.reciprocal_approx_fast(rsq, rsq)
                nc.vector.reciprocal_approx_fast(rsk, rsk)
                rqb = rqbp.tile([128, 512], FP32, tag=f"rqb{t % 2}")
                nc.gpsimd.partition_broadcast(rqb, rsq)
                rkb = rqbp.tile([128, 512], FP32, tag=f"rkb{t % 2}")
                nc.gpsimd.partition_broadcast(rkb, rsk)
                rqbs[t] = rqb
                rkbs[t] = rkb

        def emit_rope(t):
            qkvt, _ = qkv_tiles.pop(t)
            rqb = rqbs.pop(t)
            rkb = rkbs.pop(t)
            tc_sl = ts(t, 512)
            for h in range(5):  # 4 q heads + k
                swps = mm.tile([128, 512], FP32, tag="mm")
                nc.tensor.matmul(swps, pm, qkvt[:, h, :],
                                 start=True, stop=True)
                t1 = ropep.tile([128, 512], BF, tag="t1")
                t2 = ropep.tile([128, 512], BF, tag="t2")
                nc.vector.tensor_mul(t1, qkvt[:, h, :], cs_sb[:, tc_sl])
                nc.vector.tensor_mul(t2, swps, sn_sb[:, tc_sl])
                nc.vector.tensor_add(t1, t1, t2)
                if h < 4:
                    nc.vector.tensor_mul(qhat[:, h, tc_sl], t1, rqb)
                else:
                    nc.vector.tensor_mul(khat[:, tc_sl], t1, rkb)

        # ========= attention =========
        # Per step: scores+exp+mask+den-fold for (g,h) interleaved with AV
        # matmuls of the previous head at pair granularity so the PE always
        # has ready work adjacent to each exp-gated score pair.
        yhats = {}
        pending = []   # one staged (g, h, pts, dacc) awaiting AV emission

        def emit_da_tail(g, h, yp, dacc):
            # den matmul (single, over the pair-folded sums) then 1/den on
            # DVE (custom NR op; SBUF operands) and the yhat scale.
            dp = aux.tile([1, 256], FP32, tag="aux")
            nc.tensor.matmul(dp, ones[:, 0:1], dacc, start=True, stop=True)
            yhat = yhats[g // 2]
            gc = slice(256 * (g % 2), 256 * (g % 2) + 256)
            ld = dsbp.tile([1, 256], FP32, tag="ld")
            nc.scalar.copy(ld, dp)
            rc = dsbp.tile([1, 256], FP32, tag="rc")
            nc.vector.reciprocal_approx_fast(rc, ld)
            drb = drbp.tile([128, 256], FP32, tag="drb")
            nc.gpsimd.partition_broadcast(drb, rc)
            nc.vector.tensor_mul(yhat[:, h, gc], yp, drb)

        def attn_step(g, h):
            """Emit scores for (g,h) [if not None] + AV for pending."""
            cur = []
            dacc = None
            npair_cur = 0
            if g is not None:
                jlo = max(0, 2 * g - 8)
                jhi = 2 * g + 1
                cur_js = list(range(jlo, jhi, 2))
                npair_cur = len(cur_js)
                dacc = daccp.tile([128, 256], BF, tag="dacc")
            prev = pending.pop() if pending else None
            if prev is not None:
                pg, ph, ppts, pdacc = prev
                pjlo = ppts[0][0]
                pjhi = ppts[-1][0] + 1
                yp = yps.tile([128, 256], FP32, tag="y")
            nsteps = max(npair_cur, len(ppts) if prev else 0)
            for idx in range(nsteps):
                if idx < npair_cur:
                    j = cur_js[idx]
                    sc = mm.tile([128, 512], FP32, tag="mm")
                    nc.tensor.matmul(sc[:, 0:256], khat[:, ts(j, 128)],
                                     qhat[:, h, ts(g, 256)],
                                     start=True, stop=True)
                    nc.tensor.matmul(sc[:, 256:512], khat[:, ts(j + 1, 128)],
                                     qhat[:, h, ts(g, 256)],
                                     start=True, stop=True)
                    pt = ppool.tile([128, 512], BF, tag="p")
                    nc.scalar.activation(pt, sc,
                                         mybir.ActivationFunctionType.Exp,
                                         bias=zerob)
                    if j == 2 * g:
                        nc.vector.tensor_mul(pt, pt, mask_diag)
                    elif j == jlo and g >= 4:
                        nc.vector.tensor_mul(pt, pt, mask_tail)
                    # den pair-fold on DVE
                    if idx == 0:
                        nc.vector.tensor_add(dacc, pt[:, 0:256], pt[:, 256:512])
                    else:
                        nc.vector.tensor_add(dacc, dacc, pt[:, 0:256])
                        nc.vector.tensor_add(dacc, dacc, pt[:, 256:512])
                    cur.append((j, pt))
                if prev is not None and idx < len(ppts):
                    j, pt = ppts[idx]
                    for k in range(2):
                        jj = j + k
                        sl = slice(256 * k, 256 * k + 256)
                        nc.tensor.matmul(yp, v_sb[:, jj, :], pt[:, sl],
                                         start=(jj == pjlo), stop=(jj == pjhi))
            if prev is not None:
                emit_da_tail(pg, ph, yp, pdacc)
            if g is not None:
                pending.append((g, h, cur, dacc))

        def attn_group(g):
            if g % 2 == 0:
                yhat = yhatp.tile([128, HL, 512], BF, tag="yhat")
                yhats[g // 2] = yhat
            for h in range(HL):
                attn_step(g, h)

        def flush_attn():
            attn_step(None, None)

        # ========= out-proj =========
        def emit_proj(p):
            yhat = yhats.pop(p)
            ostg = None
            for o in range(32):
                if o % 2 == 0:
                    ostg = ostgp.tile([128, 2, 512], BF, tag="ostg")
                ps = mm.tile([128, 512], FP32, tag="mm")
                for cch in range(4):
                    nc.tensor.matmul(ps, wp_sb[:, cch, ts(o, 128)],
                                     yhat[:, cch, :],
                                     start=(cch == 0), stop=(cch == 3))
                if o % 2 == 0:
                    nc.scalar.copy(ostg[:, 0, :], ps)
                else:
                    nc.vector.tensor_copy(ostg[:, 1, :], ps)
                    deng = (nc.sync, nc.scalar)[(o // 2) % 2]
                    deng.dma_start(yo_d[o // 2, :, :, ts(p, 512)], ostg)

        # ========= emission =========
        emit_qkv(0)
        emit_epilogue(0)
        emit_qkv(1)
        emit_epilogue(1)
        emit_allreduce(0)
        emit_qkv(2)
        emit_epilogue(2)
        # wp load here: DMA queues run ahead of the PE, so any earlier and
        # these 4.2MB contend with wq/x during the startup-critical window
        nc.sync.dma_start(wp_sb[:, 0, :], wp_d[:, 0, :])
        nc.sync.dma_start(wp_sb[:, 1, :], wp_d[:, 1, :])
        emit_qkv(3)
        emit_epilogue(3)
        nc.sync.dma_start(wp_sb[:, 2, :], wp_d[:, 2, :])
        nc.sync.dma_start(wp_sb[:, 3, :], wp_d[:, 3, :])
        emit_rstd(0)
        emit_rope(0)
        emit_rope(1)
        attn_group(0)
        attn_group(1)
        emit_allreduce(1)
        emit_rstd(1)
        attn_group(2)
        attn_group(3)
        emit_rope(2)
        emit_allreduce(2)
        emit_rstd(2)
        emit_rope(3)
        emit_proj(0)
        attn_group(4)
        emit_proj(1)
        attn_group(5)
        attn_group(6)
        emit_proj(2)
        attn_group(7)
        flush_attn()
        emit_proj(3)


_PROGRAM = None


def _get_program():
    global _PROGRAM
    if _PROGRAM is None:
        _PROGRAM = build_program()
    return _PROGRAM


def make_in_maps(x, cos, sin, W_qkv, norm_q_w, norm_k_w, W_proj):
    x2 = np.asarray(x, np.float32).reshape(T, C)
    xt = np.ascontiguousarray(
        x2.T.reshape(32, 128, 4, 512).transpose(2, 1, 0, 3)).astype(BF16)
    cs = np.ascontiguousarray(np.asarray(cos, np.float32).T).astype(BF16)
    sn_f = np.asarray(sin, np.float32).T.copy()  # [128, T]
    sn_f[0:64, :] *= -1.0  # rotate-half: lower half gets -x2*sin
    sn = np.ascontiguousarray(sn_f).astype(BF16)
    Wq = np.asarray(W_qkv, np.float32)
    Wp = np.asarray(W_proj, np.float32)
    nqw = np.asarray(norm_q_w, np.float32)
    nkw = np.asarray(norm_k_w, np.float32)
    pm = np.zeros((128, 128), np.float32)
    pm[np.arange(128), (np.arange(128) + 64) % 128] = 1.0
    pm = pm.astype(BF16)
    in_maps = []
    for i in range(NCORES):
        qs = slice(512 * i, 512 * i + 512)
        ks = slice(4096 + 128 * i, 4096 + 128 * i + 128)
        vs = slice(5120 + 128 * i, 5120 + 128 * i + 128)
        wq_rows = np.concatenate([
            Wq[qs] * nqw[qs][:, None],
            Wq[ks] * nkw[128 * i:128 * i + 128][:, None],
            Wq[vs],
        ], axis=0)  # [768, 4096]
        wq_t = np.ascontiguousarray(
            wq_rows.T.reshape(32, 128, 768).transpose(1, 0, 2)).astype(BF16)
        wp_t = np.ascontiguousarray(
            Wp[:, 512 * i:512 * i + 512].T.reshape(4, 128, C)
            .transpose(1, 0, 2)).astype(BF16)
        iw2 = np.ones((128, 8), np.float32)
        qw = nqw[qs].reshape(4, 128).T  # [p, chunk]
        kw = nkw[128 * i:128 * i + 128]
        with np.errstate(divide="ignore"):
            iw2[:, 0:4] = np.where(qw != 0.0, qw, 1.0) ** -2.0
            # k row weighted 4x (so q and k share the 1/4096 mean divisor)
            # and a further 128x (folds 1/sqrt(d) into rstd_k)
            iw2[:, 4] = 512.0 * np.where(kw != 0.0, kw, 1.0) ** -2.0
        in_maps.append({
            "xt": xt, "cs": cs, "sn": sn,
            "wq": wq_t, "wp": wp_t, "iw2": iw2.astype(BF16), "pm": pm,
        })
    return in_maps


def combine_outputs(results):
    acc = np.zeros((16, 2, 128, T), np.float32)
    for r in results:
        acc += np.asarray(r["yo"], dtype=np.float32).transpose(0, 2, 1, 3)
    return np.ascontiguousarray(acc.reshape(C, T).T).reshape(1, T, C)


def kernel(x, cos, sin, W_qkv, norm_q_w, norm_k_w, W_proj):
    nc = _get_program()
    in_maps = make_in_maps(x, cos, sin, W_qkv, norm_q_w, norm_k_w, W_proj)
    res = bass_utils.run_bass_kernel_spmd(nc, in_maps,
                                          core_ids=list(range(NCORES)))
    return combine_outputs(res.results)


# revision 24
# speedup vs baseline: 1.0448x; 1.0062x over previous
"""Trainium2 Bass kernel for CausalSelfAttention (GQA + QK-RMSNorm + RoPE +
sliding-window causal attention + out-proj), tensor-parallel over 8 NeuronCores.

Sharding: core i owns q heads 4i..4i+3 and kv group i (split of the qkv output
dim and the proj input dim). The QK RMSNorm spans ALL heads (norm over the full
flattened q/k vectors), so per-core partial sums of squares are combined with a
tiny AllReduce. The proj contribution of each core is a partial sum over its
heads; partials are summed on the host.

v5 (648us -> 522us traced): deferred epilogues (all 4 qkv tiles first; the
3 sumsq AllReduces batched (0+1, 2, 3) and posted with enough program-order
slack that no engine stream wedges on a collective); a 32-byte warmup
AllReduce at t~0 that absorbs the ~100us first-collective CC-stream-init /
launch-skew cost under the qkv phase (the real ARs then run at ~15us);
512-col qkv matmuls with c-outer x piece-streaming into 6 open PSUM
accumulators; softmax denominator via DVE pair-folds + one 256-col matmul
per head-group (off the PE); single-Sqrt+NR-reciprocal rstd with 1/sqrt(d)
folded into the host-side k sumsq weights; AR sends on the sync DMA queue /
receives on gpsimd so x loads (scalar queue) never head-of-line block them;
bf16 rope intermediates; paired output stores.

Self-contained: hardcodes B=1, T=2048, C=4096, H=32, G=8, D=128, W=1024.
"""

import sys
import types
import numpy as np
import ml_dtypes

import concourse.bass as bass
import concourse.tile as tile
from concourse import bacc, mybir
from concourse import bass_utils
from concourse.bass import ts
from concourse.masks import make_identity

BF16 = ml_dtypes.bfloat16
FP32 = mybir.dt.float32
BF = mybir.dt.bfloat16

T = 2048          # tokens
C = 4096          # n_embd
D = 128           # head dim
HL = 4            # local q heads per core
NCORES = 8
EPS = 1e-5
NT = 4            # t-tiles of 512
NG = 8            # q groups of 256
NKV = 16          # kv blocks of 128
NPIECE = 8        # x pieces per t-tile (4 c-chunks each)


def _install_ntff_hook():
    """Re-register the axon NTFF profiling hook (the image lacks
    antenv.axon_hooks, so boot() degraded silently)."""
    if "antenv.axon_hooks" in sys.modules:
        return
    mod = types.ModuleType("antenv.axon_hooks")
    holder = [None]
    mod.set_axon_ntff_profile_hook = lambda h: holder.__setitem__(0, h)
    mod.get_axon_ntff_profile_hook = lambda: holder[0]
    sys.modules["antenv.axon_hooks"] = mod
    try:
        import antenv
        antenv.axon_hooks = mod
        from trn_agent_boot.trn_boot import _ntff_profile_via_ctypes
        mod.set_axon_ntff_profile_hook(
            _ntff_profile_via_ctypes("/opt/axon/libaxon_pjrt.so"))
    except Exception:
        pass


_install_ntff_hook()


def build_program():
    nc = bacc.Bacc("TRN2", target_bir_lowering=False, debug=False,
                   num_devices=NCORES)

    xt_d = nc.dram_tensor("xt", [4, 128, 32, 512], BF, kind="ExternalInput").ap()
    wq_d = nc.dram_tensor("wq", [128, 32, 768], BF, kind="ExternalInput").ap()
    wp_d = nc.dram_tensor("wp", [128, 4, C], BF, kind="ExternalInput").ap()
    cs_d = nc.dram_tensor("cs", [128, T], BF, kind="ExternalInput").ap()
    sn_d = nc.dram_tensor("sn", [128, T], BF, kind="ExternalInput").ap()
    iw2_d = nc.dram_tensor("iw2", [128, 8], BF, kind="ExternalInput").ap()
    pm_d = nc.dram_tensor("pm", [128, 128], BF, kind="ExternalInput").ap()
    yo_d = nc.dram_tensor("yo", [16, 128, 2, T], BF, kind="ExternalOutput").ap()

    with tile.TileContext(nc) as tc:
        _emit(nc, tc, xt_d, wq_d, wp_d, cs_d, sn_d, iw2_d, pm_d, yo_d)
    nc.compile()
    return nc


def _emit(nc, tc, xt_d, wq_d, wp_d, cs_d, sn_d, iw2_d, pm_d, yo_d):
    import contextlib
    ctx = contextlib.ExitStack()
    with ctx:
        # ---------------- pools ----------------
        const = ctx.enter_context(tc.tile_pool(name="const", bufs=1))
        persist = ctx.enter_context(tc.tile_pool(name="persist", bufs=1))
        xpool = ctx.enter_context(tc.tile_pool(name="xpool", bufs=5))
        qkvpool = ctx.enter_context(tc.tile_pool(name="qkvpool", bufs=4))
        qkvcopy = ctx.enter_context(tc.tile_pool(name="qkvcopy", bufs=2))
        sqpool = ctx.enter_context(tc.tile_pool(name="sqpool", bufs=2))
        stgpool = ctx.enter_context(tc.tile_pool(name="stgpool", bufs=2))
        rcv = ctx.enter_context(tc.tile_pool(name="rcv", bufs=1))
        rqbp = ctx.enter_context(tc.tile_pool(name="rqbp", bufs=1))
        ropep = ctx.enter_context(tc.tile_pool(name="ropep", bufs=1))
        ppool = ctx.enter_context(tc.tile_pool(name="ppool", bufs=9))
        daccp = ctx.enter_context(tc.tile_pool(name="daccp", bufs=2))
        dsbp = ctx.enter_context(tc.tile_pool(name="dsbp", bufs=2))
        drbp = ctx.enter_context(tc.tile_pool(name="drbp", bufs=2))
        yhatp = ctx.enter_context(tc.tile_pool(name="yhatp", bufs=2))
        ostgp = ctx.enter_context(tc.tile_pool(name="ostgp", bufs=2))
        dram = ctx.enter_context(tc.tile_pool(name="dram", bufs=2, space="DRAM"))

        # PSUM budget (8 banks, slots are bank-granular):
        #   mm    6 banks (qkv accumulators, rope-swap, scores, proj)
        #   aux   1 bank  (sumsq rows in qkv phase; den rows in attn phase)
        #   yps   1 bank  (AV accumulator)
        mm = ctx.enter_context(tc.tile_pool(name="mm", bufs=6, space="PSUM"))
        aux = ctx.enter_context(tc.tile_pool(name="aux", bufs=1, space="PSUM"))
        yps = ctx.enter_context(tc.tile_pool(name="yps", bufs=1, space="PSUM"))

        # ---------------- warmup collective ----------------
        # The first collective of the program absorbs ~100-140us of CC-stream
        # init / cross-core launch skew. Pay that cost on a 32-byte dummy
        # issued at t~0 (hidden under the qkv phase) so the real AllReduces
        # run at their ~18us steady-state latency.
        zrow = const.tile([1, 8], FP32)
        nc.vector.memset(zrow, 0.0)
        dmy_in = dram.tile([1, 8], FP32, tag="dmyi")
        dmy_out = dram.tile([1, 8], FP32, tag="dmyo")
        nc.gpsimd.dma_start(dmy_in, zrow)
        nc.gpsimd.collective_compute(
            "AllReduce", mybir.AluOpType.add,
            replica_groups=[list(range(NCORES))],
            ins=[dmy_in.opt()], outs=[dmy_out.opt()])

        # ---------------- constants (no DMA deps first) ----------------
        ident = const.tile([128, 128], BF)
        make_identity(nc, ident)
        ones = const.tile([128, 1], BF)
        nc.vector.memset(ones, 1.0)
        zerob = const.tile([128, 1], FP32)
        nc.vector.memset(zerob, 0.0)
        # sqrt bias: k rows get 128*eps (1/sqrt(d) folded into the k sumsq
        # via the host iw2 weighting)
        epsq = const.tile([1, 1], FP32)
        nc.vector.memset(epsq, EPS)
        epsk = const.tile([1, 1], FP32)
        nc.vector.memset(epsk, 128.0 * EPS)
        # window/causal edge masks, [kv=128, 2*256] (a pair of kv blocks for
        # the same 256 queries). diag pair: m in {0,1}; tail pair: m in {-8,-7}
        # where m = j - 2g. allowed iff 0 <= d <= 1023 with d = q - kv - 128*m.
        mask_diag = const.tile([128, 512], BF)
        nc.vector.memset(mask_diag, 1.0)
        mask_tail = const.tile([128, 512], BF)
        nc.vector.memset(mask_tail, 1.0)
        for half, m in ((0, 0), (1, 1)):
            nc.gpsimd.affine_select(
                out=mask_diag[:, 256 * half:256 * half + 256],
                in_=mask_diag[:, 256 * half:256 * half + 256],
                compare_op=mybir.AluOpType.is_ge,
                fill=0.0, base=-128 * m, pattern=[[1, 256]],
                channel_multiplier=-1)
        for half, m in ((0, -8), (1, -7)):
            nc.gpsimd.affine_select(
                out=mask_tail[:, 256 * half:256 * half + 256],
                in_=mask_tail[:, 256 * half:256 * half + 256],
                compare_op=mybir.AluOpType.is_ge,
                fill=0.0, base=128 * m + 1023, pattern=[[-1, 256]],
                channel_multiplier=1)

        # small input consts on sync queue (before wq so they land early)
        iw2 = const.tile([128, 8], BF)
        nc.sync.dma_start(iw2, iw2_d)
        pm = const.tile([128, 128], BF)
        nc.sync.dma_start(pm, pm_d)

        # ---------------- persistent buffers ----------------
        # wq on the sync DMA queue; x pieces go on the scalar queue so the
        # gpsimd sequencer (which hosts the collective triggers) stays empty.
        wq_sb = persist.tile([128, 32, 768], BF)
        for h in range(8):
            nc.sync.dma_start(wq_sb[:, ts(h, 4), :], wq_d[:, ts(h, 4), :])
        wp_sb = persist.tile([128, 4, C], BF)   # loaded later (deferred)

        # cos/sin full tables early on the gpsimd queue (it is otherwise idle
        # until the first collective trigger)
        cs_sb = persist.tile([128, T], BF)
        nc.gpsimd.dma_start(cs_sb, cs_d)
        sn_sb = persist.tile([128, T], BF)
        nc.gpsimd.dma_start(sn_sb, sn_d)

        qhat = persist.tile([128, HL, T], BF)     # roped+normed q
        khat = persist.tile([128, T], BF)         # roped+normed k (1/sqrt(d) in)
        v_sb = persist.tile([128, NKV, 128], BF)  # v transposed [kv, d]

        qkv_tiles = {}
        arins = {}
        arouts = {}
        # AR batches: tiles (0,1) together, then 2 and 3 alone so their rstd
        # unblocks as soon as that tile's sumsq lands
        BATCHES = ((0, 1), (2,), (3,))
        TILE_BATCH = {0: (0, 0), 1: (0, 1), 2: (1, 0), 3: (2, 0)}

        # ========= qkv tile: c-outer, 6 open PSUM accumulators =========
        def emit_qkv(t):
            qkvt = qkvpool.tile([128, 5, 512], BF, tag="qkvt")
            vstage = qkvcopy.tile([128, 512], BF, tag="vstage")
            qkv_tiles[t] = (qkvt, vstage)
            ps = []
            for _o in range(6):
                pso = mm.tile([128, 512], FP32, tag="mm")
                ps.append(pso)
            for p in range(NPIECE):
                xp = xpool.tile([128, 4, 512], BF, tag="x")
                nc.scalar.dma_start(xp, xt_d[t, :, ts(p, 4), :])
                for o in range(6):
                    for ci in range(4):
                        c = 4 * p + ci
                        nc.tensor.matmul(ps[o],
                                         wq_sb[:, c, ts(o, 128)],
                                         xp[:, ci, :],
                                         start=(c == 0), stop=(c == 31))
            for o in range(6):
                if o < 5:
                    nc.scalar.copy(qkvt[:, o, :], ps[o])
                else:
                    nc.scalar.copy(vstage, ps[o])

        # ========= epilogue: v transpose + weighted sumsq + AR send =========
        def emit_epilogue(t):
            qkvt, vstage = qkv_tiles[t]
            # v transpose: [d, kv] -> [kv, d]
            for bb in range(4):
                tps = mm.tile([128, 128], BF, tag="mm")
                nc.tensor.transpose(tps, vstage[:, ts(bb, 128)], ident)
                nc.vector.tensor_copy(v_sb[:, 4 * t + bb, :], tps)
            # sum of squares (weighted by 1/w^2; k row weighted 4*128x on host:
            # 4x so q/k share the 1/4096 mean divisor, 128x folds 1/sqrt(d)
            # into rstd_k). row 0 = q, row 32 = k.
            sums = aux.tile([33, 512], FP32, tag="aux")
            for cch in range(4):
                sq = sqpool.tile([128, 512], BF, tag="sq")
                nc.vector.tensor_mul(sq, qkvt[:, cch, :], qkvt[:, cch, :])
                nc.tensor.matmul(sums[0:1, :], iw2[:, cch:cch + 1], sq,
                                 start=(cch == 0), stop=(cch == 3))
            sqk = sqpool.tile([128, 512], BF, tag="sq")
            nc.vector.tensor_mul(sqk, qkvt[:, 4, :], qkvt[:, 4, :])
            nc.tensor.matmul(sums[32:33, :], iw2[:, 4:5], sqk,
                             start=True, stop=True)
            stg = stgpool.tile([33, 512], FP32, tag="stg")
            nc.vector.tensor_copy(stg[0:1, :], sums[0:1, :])
            nc.vector.tensor_copy(stg[32:33, :], sums[32:33, :])
            # stage rows [q(t), k(t)] of this AR batch's input
            b, slot = TILE_BATCH[t]
            nrows = 2 * len(BATCHES[b])
            if slot == 0:
                arin = dram.tile([nrows, 512], FP32, tag=f"arin{b}")
                arout = dram.tile([nrows, 512], FP32, tag=f"arout{b}")
                arins[b] = arin
                arouts[b] = arout
            # AR sends ride the sync queue: its sequencer is idle mid-kernel,
            # and the wp loads emitted between sends are not latency-critical
            nc.sync.dma_start(arins[b][2 * slot:2 * slot + 1, :], stg[0:1, :])
            nc.sync.dma_start(arins[b][2 * slot + 1:2 * slot + 2, :],
                              stg[32:33, :])

        def emit_allreduce(b):
            nc.gpsimd.collective_compute(
                "AllReduce", mybir.AluOpType.add,
                replica_groups=[list(range(NCORES))],
                ins=[arins[b].opt()], outs=[arouts[b].opt()])

        # ========= ep2: rstd + rope for one AR batch =========
        rqbs = {}
        rkbs = {}

        def emit_rstd(b):
            # rstd = 1/sqrt(ss/4096 + eps); k rows come host-scaled by 128 so
            # this is rstd_k/sqrt(d) with bias 128*eps. (Rsqrt ACT is
            # blocklisted, so Sqrt + the DVE fast-NR reciprocal. Per-row
            # [1,512] tiles: SBUF APs must start at partition 0/32/64/96.)
            for slot, t in enumerate(BATCHES[b]):
                rsq = rcv.tile([1, 512], FP32, tag="rsq")
                nc.gpsimd.dma_start(rsq, arouts[b][2 * slot:2 * slot + 1, :])
                rsk = rcv.tile([1, 512], FP32, tag="rsk")
                nc.gpsimd.dma_start(rsk,
                                    arouts[b][2 * slot + 1:2 * slot + 2, :])
                nc.scalar.activation(rsq, rsq,
                                     mybir.ActivationFunctionType.Sqrt,
                                     bias=epsq, scale=1.0 / 4096.0)
                nc.scalar.activation(rsk, rsk,
                                     mybir.ActivationFunctionType.Sqrt,
                                     bias=epsk, scale=1.0 / 4096.0)
                nc.vector.reciprocal_approx_fast(rsq, rsq)
                nc.vector.reciprocal_approx_fast(rsk, rsk)
                rqb = rqbp.tile([128, 512], FP32, tag=f"rqb{t % 2}")
                nc.gpsimd.partition_broadcast(rqb, rsq)
                rkb = rqbp.tile([128, 512], FP32, tag=f"rkb{t % 2}")
                nc.gpsimd.partition_broadcast(rkb, rsk)
                rqbs[t] = rqb
                rkbs[t] = rkb

        def emit_rope(t):
            qkvt, _ = qkv_tiles.pop(t)
            rqb = rqbs.pop(t)
            rkb = rkbs.pop(t)
            tc_sl = ts(t, 512)
            for h in range(5):  # 4 q heads + k
                swps = mm.tile([128, 512], FP32, tag="mm")
                nc.tensor.matmul(swps, pm, qkvt[:, h, :],
                                 start=True, stop=True)
                t1 = ropep.tile([128, 512], BF, tag="t1")
                t2 = ropep.tile([128, 512], BF, tag="t2")
                nc.vector.tensor_mul(t1, qkvt[:, h, :], cs_sb[:, tc_sl])
                nc.vector.tensor_mul(t2, swps, sn_sb[:, tc_sl])
                nc.vector.tensor_add(t1, t1, t2)
                if h < 4:
                    nc.vector.tensor_mul(qhat[:, h, tc_sl], t1, rqb)
                else:
                    nc.vector.tensor_mul(khat[:, tc_sl], t1, rkb)

        # ========= attention =========
        # Per step: scores+exp+mask+den-fold for (g,h) interleaved with AV
        # matmuls of the previous head at pair granularity so the PE always
        # has ready work adjacent to each exp-gated score pair.
        yhats = {}
        pending = []   # one staged (g, h, pts, dacc) awaiting AV emission

        def emit_da_tail(g, h, yp, dacc):
            # den matmul (single, over the pair-folded sums) then 1/den on
            # DVE (custom NR op; SBUF operands) and the yhat scale.
            dp = aux.tile([1, 256], FP32, tag="aux")
            nc.tensor.matmul(dp, ones[:, 0:1], dacc, start=True, stop=True)
            yhat = yhats[g // 2]
            gc = slice(256 * (g % 2), 256 * (g % 2) + 256)
            # DVE copy: an ACT copy here head-of-line blocks the exp stream
            # behind the den-matmul wait (~2-3us per head-group)
            ld = dsbp.tile([1, 256], FP32, tag="ld")
            nc.vector.tensor_copy(ld, dp)
            rc = dsbp.tile([1, 256], FP32, tag="rc")
            nc.vector.reciprocal_approx_fast(rc, ld)
            drb = drbp.tile([128, 256], FP32, tag="drb")
            nc.gpsimd.partition_broadcast(drb, rc)
            nc.vector.tensor_mul(yhat[:, h, gc], yp, drb)

        def attn_step(g, h):
            """Emit scores for (g,h) [if not None] + AV for pending."""
            cur = []
            dacc = None
            npair_cur = 0
            if g is not None:
                jlo = max(0, 2 * g - 8)
                jhi = 2 * g + 1
                cur_js = list(range(jlo, jhi, 2))
                npair_cur = len(cur_js)
                dacc = daccp.tile([128, 256], BF, tag="dacc")
            prev = pending.pop() if pending else None
            if prev is not None:
                pg, ph, ppts, pdacc = prev
                pjlo = ppts[0][0]
                pjhi = ppts[-1][0] + 1
                yp = yps.tile([128, 256], FP32, tag="y")
            nsteps = max(npair_cur, len(ppts) if prev else 0)
            for idx in range(nsteps):
                if idx < npair_cur:
                    j = cur_js[idx]
                    sc = mm.tile([128, 512], FP32, tag="mm")
                    nc.tensor.matmul(sc[:, 0:256], khat[:, ts(j, 128)],
                                     qhat[:, h, ts(g, 256)],
                                     start=True, stop=True)
                    nc.tensor.matmul(sc[:, 256:512], khat[:, ts(j + 1, 128)],
                                     qhat[:, h, ts(g, 256)],
                                     start=True, stop=True)
                    pt = ppool.tile([128, 512], BF, tag="p")
                    nc.scalar.activation(pt, sc,
                                         mybir.ActivationFunctionType.Exp,
                                         bias=zerob)
                    if j == 2 * g:
                        nc.vector.tensor_mul(pt, pt, mask_diag)
                    elif j == jlo and g >= 4:
                        nc.vector.tensor_mul(pt, pt, mask_tail)
                    # den pair-fold on DVE
                    if idx == 0:
                        nc.vector.tensor_add(dacc, pt[:, 0:256], pt[:, 256:512])
                    else:
                        nc.vector.tensor_add(dacc, dacc, pt[:, 0:256])
                        nc.vector.tensor_add(dacc, dacc, pt[:, 256:512])
                    cur.append((j, pt))
                if prev is not None and idx < len(ppts):
                    j, pt = ppts[idx]
                    for k in range(2):
                        jj = j + k
                        sl = slice(256 * k, 256 * k + 256)
                        nc.tensor.matmul(yp, v_sb[:, jj, :], pt[:, sl],
                                         start=(jj == pjlo), stop=(jj == pjhi))
            if prev is not None:
                emit_da_tail(pg, ph, yp, pdacc)
            if g is not None:
                pending.append((g, h, cur, dacc))

        def attn_group(g):
            if g % 2 == 0:
                yhat = yhatp.tile([128, HL, 512], BF, tag="yhat")
                yhats[g // 2] = yhat
            for h in range(HL):
                attn_step(g, h)

        def flush_attn():
            attn_step(None, None)

        # ========= out-proj =========
        def emit_proj(p):
            yhat = yhats.pop(p)
            ostg = None
            for o in range(32):
                if o % 2 == 0:
                    ostg = ostgp.tile([128, 2, 512], BF, tag="ostg")
                ps = mm.tile([128, 512], FP32, tag="mm")
                for cch in range(4):
                    nc.tensor.matmul(ps, wp_sb[:, cch, ts(o, 128)],
                                     yhat[:, cch, :],
                                     start=(cch == 0), stop=(cch == 3))
                if o % 2 == 0:
                    nc.scalar.copy(ostg[:, 0, :], ps)
                else:
                    nc.vector.tensor_copy(ostg[:, 1, :], ps)
                    deng = (nc.sync, nc.scalar)[(o // 2) % 2]
                    deng.dma_start(yo_d[o // 2, :, :, ts(p, 512)], ostg)

        # ========= emission =========
        emit_qkv(0)
        emit_epilogue(0)
        emit_qkv(1)
        emit_epilogue(1)
        emit_allreduce(0)
        emit_qkv(2)
        emit_epilogue(2)
        # wp load here: DMA queues run ahead of the PE, so any earlier and
        # these 4.2MB contend with wq/x during the startup-critical window
        nc.sync.dma_start(wp_sb[:, 0, :], wp_d[:, 0, :])
        nc.sync.dma_start(wp_sb[:, 1, :], wp_d[:, 1, :])
        emit_qkv(3)
        emit_epilogue(3)
        nc.sync.dma_start(wp_sb[:, 2, :], wp_d[:, 2, :])
        nc.sync.dma_start(wp_sb[:, 3, :], wp_d[:, 3, :])
        emit_rstd(0)
        emit_rope(0)
        emit_rope(1)
        attn_group(0)
        attn_group(1)
        emit_allreduce(1)
        emit_rstd(1)
        attn_group(2)
        attn_group(3)
        emit_rope(2)
        emit_allreduce(2)
        emit_rstd(2)
        emit_rope(3)
        emit_proj(0)
        attn_group(4)
        emit_proj(1)
        attn_group(5)
        attn_group(6)
        emit_proj(2)
        attn_group(7)
        flush_attn()
        emit_proj(3)


_PROGRAM = None


def _get_program():
    global _PROGRAM
    if _PROGRAM is None:
        _PROGRAM = build_program()
    return _PROGRAM


def make_in_maps(x, cos, sin, W_qkv, norm_q_w, norm_k_w, W_proj):
    x2 = np.asarray(x, np.float32).reshape(T, C)
    xt = np.ascontiguousarray(
        x2.T.reshape(32, 128, 4, 512).transpose(2, 1, 0, 3)).astype(BF16)
    cs = np.ascontiguousarray(np.asarray(cos, np.float32).T).astype(BF16)
    sn_f = np.asarray(sin, np.float32).T.copy()  # [128, T]
    sn_f[0:64, :] *= -1.0  # rotate-half: lower half gets -x2*sin
    sn = np.ascontiguousarray(sn_f).astype(BF16)
    Wq = np.asarray(W_qkv, np.float32)
    Wp = np.asarray(W_proj, np.float32)
    nqw = np.asarray(norm_q_w, np.float32)
    nkw = np.asarray(norm_k_w, np.float32)
    pm = np.zeros((128, 128), np.float32)
    pm[np.arange(128), (np.arange(128) + 64) % 128] = 1.0
    pm = pm.astype(BF16)
    in_maps = []
    for i in range(NCORES):
        qs = slice(512 * i, 512 * i + 512)
        ks = slice(4096 + 128 * i, 4096 + 128 * i + 128)
        vs = slice(5120 + 128 * i, 5120 + 128 * i + 128)
        wq_rows = np.concatenate([
            Wq[qs] * nqw[qs][:, None],
            Wq[ks] * nkw[128 * i:128 * i + 128][:, None],
            Wq[vs],
        ], axis=0)  # [768, 4096]
        wq_t = np.ascontiguousarray(
            wq_rows.T.reshape(32, 128, 768).transpose(1, 0, 2)).astype(BF16)
        wp_t = np.ascontiguousarray(
            Wp[:, 512 * i:512 * i + 512].T.reshape(4, 128, C)
            .transpose(1, 0, 2)).astype(BF16)
        iw2 = np.ones((128, 8), np.float32)
        qw = nqw[qs].reshape(4, 128).T  # [p, chunk]
        kw = nkw[128 * i:128 * i + 128]
        with np.errstate(divide="ignore"):
            iw2[:, 0:4] = np.where(qw != 0.0, qw, 1.0) ** -2.0
            # k row weighted 4x (so q and k share the 1/4096 mean divisor)
            # and a further 128x (folds 1/sqrt(d) into rstd_k)
            iw2[:, 4] = 512.0 * np.where(kw != 0.0, kw, 1.0) ** -2.0
        in_maps.append({
            "xt": xt, "cs": cs, "sn": sn,
            "wq": wq_t, "wp": wp_t, "iw2": iw2.astype(BF16), "pm": pm,
        })
    return in_maps


def combine_outputs(results):
    acc = np.zeros((16, 2, 128, T), np.float32)
    for r in results:
        acc += np.asarray(r["yo"], dtype=np.float32).transpose(0, 2, 1, 3)
    return np.ascontiguousarray(acc.reshape(C, T).T).reshape(1, T, C)


def kernel(x, cos, sin, W_qkv, norm_q_w, norm_k_w, W_proj):
    nc = _get_program()
    in_maps = make_in_maps(x, cos, sin, W_qkv, norm_q_w, norm_k_w, W_proj)
    res = bass_utils.run_bass_kernel_spmd(nc, in_maps,
                                          core_ids=list(range(NCORES)))
    return combine_outputs(res.results)
